# revision 47
# baseline (speedup 1.0000x reference)
"""Trainium2 Bass kernel for nn_ConsistencyLoss.

Strategy (pure data-parallel over the agent dim N, 8 cores):
  - Host pads N 20000 -> 20480, shards 2560 agents/core, and builds:
      * tds: per-block [38, 128] f16 stationary tensors holding quantized
        endpoint distances (f16(1024 + 64*dist), exact grid-1 integers)
        plus two ones-rows for the offset-cancel and index-payload terms
      * lp/ln trajectories in a tc-major "gather layout" (bf16): each
        16-partition group owns 320 agents, partitions within a group are
        timestep slots, so the gpsimd indirect-copy gather (whose index
        list is shared across a 16-partition group) can select modes
        per-agent.
  - Match path on device: per-agent scores for all 720 mode permutations
    via two PE matmuls per 128-agent block against a [38, 720] table whose
    rows are -S/64 | +96 | (64+720-p)*2^-17.  Scores are exact f32 on a
    2^-17 grid, so a single vector.max finds the best permutation AND its
    index (packed in the low bits; extracted with f32 offset-rounding).
  - Index -> permutation images via branchless Lehmer decode (tiny ops),
    then chunked gpsimd indirect_copy gathers the selected trajectories.
  - smooth-L1 sums via 3 ACT passes with fused accumulation, using
    sum smooth(d) = sum|d| - 0.5*N + 0.5*sum relu(1-|d|)^2; zero rows
    (padding / masked agents) contribute exactly 0.
  - 5 block-segments chase each other through match -> decode -> gather ->
    smooth so all engines overlap.
  - The reg loss depends only on pad_loc/pad_loc_target (0.2% of the
    input bytes) and is computed on the host.

Self-contained: hardcodes shapes/sharding; only needs /opt/trn_rl_repo.
"""

import sys
from itertools import permutations

import numpy as np

if "/opt/trn_rl_repo" not in sys.path:
    sys.path.insert(0, "/opt/trn_rl_repo")

NUM_MODES = 6
T = 30
NPERM = 720
N_CORES = 8
PPART = 128
TC = 64  # t*2 coords padded 60 -> 64 (16 slots of 4)

PERMS = np.array(list(permutations(range(NUM_MODES))), dtype=np.int64)  # [720, 6]

EXT_C = 786432.0  # 1.5*2^19: extraction offset (grid 2^-4 over [2^19,2^20))


def _bf16_pack(x):
    """f32 array -> uint16 bf16 (RNE)."""
    x = np.ascontiguousarray(x, np.float32)
    u = x.view(np.uint32)
    r = ((u >> 16) + ((u >> 15) & 1)).astype(np.uint32)
    return (r & 0xFFFF).astype(np.uint16)


def _host_negs():
    """[38, 720] f16 table: rows 0-35 -S/16, row 36 offset-cancel, row 37 a
    payload encoding (sigma0, sigma1, lehmer d2, d3, d4) of each permutation
    in the low-order score bits."""
    negs = np.zeros((38, NPERM), np.float16)
    for p in range(NPERM):
        for i in range(NUM_MODES):
            negs[i * 6 + PERMS[p, i], p] = np.float16(-1.0 / 16.0)
    negs[36, :] = np.float16(384.0)
    pr = np.arange(NPERM)
    d0 = pr // 120
    r = pr - 120 * d0
    d1 = r // 24
    r = r - 24 * d1
    d2 = r // 6
    r = r - 6 * d2
    d3 = r // 2
    d4 = r - 2 * d3
    k = PERMS[:, 0] * 256 + PERMS[:, 1] * 32 + d2 * 8 + d3 * 2 + d4
    negs[37, :] = ((512.0 + k) * 2.0 ** -16).astype(np.float16)
    return negs


def build_nc(nsh):
    """Per-core Bass program for a shard of `nsh` agents (nsh % 256 == 0)."""
    import concourse.bacc as bacc
    import concourse.mybir as mybir
    import concourse.tile as tile

    f32 = mybir.dt.float32
    f16 = mybir.dt.float16
    bf16 = mybir.dt.bfloat16
    u16 = mybir.dt.uint16
    i32 = mybir.dt.int32
    Alu = mybir.AluOpType
    Act = mybir.ActivationFunctionType

    A = nsh // PPART
    assert A % 2 == 0
    G = 16 * A  # agents per 16-partition group
    FREE = G * NUM_MODES * 4  # bf16 elems per partition in gather layout

    nc = bacc.Bacc(None, target_bir_lowering=False, debug=False)

    # f16/bf16 payloads are shipped as f32-typed words (bitcast on SBUF side)
    td_d = nc.declare_dram_parameter("tds", [38, A * PPART // 2], f32, False)
    ng_d = nc.declare_dram_parameter("negs", [38, NPERM // 2], f32, False)
    cc_d = nc.declare_dram_parameter("cconst", [PPART, A], f32, False)
    lnT_d = nc.declare_dram_parameter("lnT", [PPART, FREE // 2], f32, False)
    lpT_d = nc.declare_dram_parameter("lpT", [PPART, FREE // 2], f32, False)
    out_d = nc.declare_dram_parameter("partials", [PPART, 10], f32, True)

    with tile.TileContext(nc) as tc:
        with (
            tc.tile_pool(name="cst", bufs=1) as cst,
            tc.tile_pool(name="big", bufs=1) as big,
            tc.tile_pool(name="sml", bufs=1) as sml,
            tc.tile_pool(name="pnm", bufs=3, space="PSUM") as pnm,
        ):
            # ---- small inputs (match-path first: they gate everything) ----
            tds = cst.tile([38, A, PPART], f16)
            nc.sync.dma_start(
                tds[:].rearrange("c a p -> c (a p)").bitcast(f32), td_d[:]
            )
            negs = cst.tile([38, NPERM], f16)
            nc.sync.dma_start(negs[:].bitcast(f32), ng_d[:])
            cconst = cst.tile([PPART, A], f32)
            nc.sync.dma_start(cconst[:], cc_d[:])

            # ---- big trajectory tensors (gather layout, bf16), per half ----
            lnT = big.tile([PPART, G * NUM_MODES, 4], bf16)
            lpT = big.tile([PPART, G * NUM_MODES, 4], bf16)
            for h in (0, 1):
                nc.sync.dma_start(
                    lnT[:, h * (G * NUM_MODES // 2) : (h + 1) * (G * NUM_MODES // 2), :]
                    .rearrange("p a b -> p (a b)")
                    .bitcast(f32),
                    lnT_d[:, h * (FREE // 4) : (h + 1) * (FREE // 4)],
                )
            for h in (0, 1):
                nc.sync.dma_start(
                    lpT[:, h * (G * NUM_MODES // 2) : (h + 1) * (G * NUM_MODES // 2), :]
                    .rearrange("p a b -> p (a b)")
                    .bitcast(f32),
                    lpT_d[:, h * (FREE // 4) : (h + 1) * (FREE // 4)],
                )

            partials = sml.tile([PPART, 10], f32)
            nc.gpsimd.memset(partials[:], 0.0)
            sel = big.tile([PPART, G * NUM_MODES, 4], bf16)
            dd = big.tile([PPART, FREE], bf16)
            ab = big.tile([PPART, FREE], bf16)
            rt = big.tile([PPART, FREE], bf16)
            idx = sml.tile([PPART, A * 6], u16)

            def match_block(a, mseg, aoff):
                nm = pnm.tile([PPART, NPERM], f32, tag="nm")
                nc.tensor.matmul(nm[:, 0:512], tds[:, a, :], negs[:, 0:512])
                nc.tensor.matmul(nm[:, 512:NPERM], tds[:, a, :], negs[:, 512:NPERM])
                nc.vector.max(mseg[:, a - aoff, :], nm[:])

            def decode(lo_, hi_, k, mseg, eng=None):
                """Payload extraction + short Lehmer adjust for [lo_, hi_).

                The max value carries (sigma0, sigma1, d2, d3, d4) packed in
                its low bits.  Bit fields are extracted on DVE (i32, the only
                engine/width with shifts), the insert-sort adjustment runs in
                f32 on `eng` so early segments can use the idle Pool engine.
                """
                if eng is None:
                    eng = nc.vector
                L = hi_ - lo_
                vm = mseg[:, :, 0:1].rearrange("p a x -> p (a x)")
                sig = sml.tile([PPART, L, 6], f32, name=f"sig{k}")

                c1 = sml.tile([PPART, L], f32, name=f"c1_{k}")
                nc.vector.tensor_scalar(c1[:], vm, EXT_C, None, Alu.add)
                negio = sml.tile([PPART, L], f32, name=f"negio{k}")
                nc.vector.scalar_tensor_tensor(
                    negio[:], c1[:], EXT_C, vm, Alu.subtract, Alu.subtract
                )
                nf = sml.tile([PPART, L], i32, name=f"nf{k}")
                nc.vector.tensor_scalar(
                    nf[:], negio[:], -65536.0, -512.0, Alu.mult, Alu.add
                )

                def sslice(i):
                    return sig[:, :, i : i + 1].rearrange("p a x -> p (a x)")

                # bit fields: i32-only on DVE (TSP bitVec ops cannot cast)
                fi = sml.tile([PPART, L, 5], i32, name=f"fi{k}")
                for j, (shift, mask) in enumerate(
                    ((8, 7), (5, 7), (3, 3), (1, 3), (0, 1))
                ):
                    nc.vector.tensor_scalar(
                        fi[:, :, j], nf[:], shift, mask,
                        Alu.logical_shift_right, Alu.bitwise_and,
                    )
                ff = sml.tile([PPART, L, 5], f32, name=f"ff{k}")
                nc.vector.tensor_copy(ff[:], fi[:])
                nc.vector.tensor_copy(sig[:, :, 0:2], ff[:, :, 0:2])
                s0 = ff[:, :, 0:1].rearrange("p a x -> p (a x)")
                s1 = ff[:, :, 1:2].rearrange("p a x -> p (a x)")
                d2 = ff[:, :, 2:3].rearrange("p a x -> p (a x)")
                d3 = ff[:, :, 3:4].rearrange("p a x -> p (a x)")
                d4 = ff[:, :, 4:5].rearrange("p a x -> p (a x)")

                def tt(op, x, y, nm_, out=None):
                    if out is None:
                        r = sml.tile([PPART, L], f32, name=f"{nm_}_{k}")
                        out = r[:]
                    eng.tensor_tensor(out, x, y, op)
                    return out

                def geadd(v, sv, nm_, out=None):
                    ge = tt(Alu.is_ge, v, sv, nm_ + "g")
                    return tt(Alu.add, v, ge, nm_ + "a", out=out)

                lo01 = tt(Alu.min, s0, s1, "lo")
                hi01 = tt(Alu.max, s0, s1, "hi")
                v2 = geadd(d2, lo01, "v2")
                s2 = geadd(v2, hi01, "s2", out=sslice(2))
                u1 = tt(Alu.min, s2, lo01, "u1")
                u2 = tt(Alu.max, s2, lo01, "u2")
                mid = tt(Alu.min, u2, hi01, "mid")
                hi3 = tt(Alu.max, u2, hi01, "hi3")
                v3 = geadd(d3, u1, "v3a")
                v3 = geadd(v3, mid, "v3b")
                s3 = geadd(v3, hi3, "s3", out=sslice(3))
                w1 = tt(Alu.min, s3, u1, "w1")
                w2p = tt(Alu.max, s3, u1, "w2p")
                w2 = tt(Alu.min, w2p, mid, "w2")
                w3p = tt(Alu.max, w2p, mid, "w3p")
                w3 = tt(Alu.min, w3p, hi3, "w3")
                w4 = tt(Alu.max, w3p, hi3, "w4")
                v4 = geadd(d4, w1, "v4a")
                v4 = geadd(v4, w2, "v4b")
                v4 = geadd(v4, w3, "v4c")
                s4 = geadd(v4, w4, "s4", out=sslice(4))
                a5 = tt(Alu.add, s0, s1, "a5a")
                b5 = tt(Alu.add, s2, s3, "a5b")
                a5 = tt(Alu.add, a5, s4, "a5c")
                a5 = tt(Alu.add, a5, b5, "a5d")
                nc.vector.tensor_scalar(
                    sslice(5), a5, -1.0, 15.0, Alu.mult, Alu.add
                )

                # gather index values: 4*sigma + cconst (chunk-local base)
                nc.vector.scalar_tensor_tensor(
                    idx[:, lo_ * 6 : hi_ * 6].rearrange("p (a x) -> p a x", x=6),
                    sig[:],
                    4.0,
                    cconst[:, lo_:hi_].unsqueeze(2).broadcast_to([PPART, L, 6]),
                    Alu.mult,
                    Alu.add,
                )

            def gather(a_lo, a_hi):
                # ISA: <=1024 dst elems/partition per IndirectCopy, 4B-aligned
                # index address -> 2-block chunks, chunk-local data slices.
                for a0 in range(a_lo, a_hi, 2):
                    m0 = a0 * 96
                    nc.gpsimd.indirect_copy(
                        sel[:, m0 : m0 + 192, :],
                        lnT[:, m0 : m0 + 192, :],
                        idx[:, 6 * a0 : 6 * a0 + 12],
                        True,
                    )

            def smooth(a_lo, a_hi, acc, sub_eng=None):
                if sub_eng is None:
                    sub_eng = nc.vector
                e_ = slice(a_lo * 96 * 4, a_hi * 96 * 4)
                m_ = slice(a_lo * 96, a_hi * 96)
                sub_eng.tensor_sub(
                    dd[:, e_],
                    lpT[:, m_, :].rearrange("p a b -> p (a b)"),
                    sel[:, m_, :].rearrange("p a b -> p (a b)"),
                )
                nc.scalar.activation(
                    ab[:, e_], dd[:, e_], Act.Abs, bias=0.0,
                    accum_out=partials[:, acc : acc + 1],
                )
                nc.scalar.activation(
                    rt[:, e_], ab[:, e_], Act.Relu, bias=1.0, scale=-1.0
                )
                nc.scalar.activation(
                    dd[:, e_], rt[:, e_], Act.Square, bias=0.0,
                    accum_out=partials[:, acc + 1 : acc + 2],
                )

            # ---- pipelined issue order: segments chase each other ----
            q = ((A // 4) + 1) & ~1
            bounds = [0, q, A // 2, A // 2 + q, A]
            if A == 20:
                bounds = [0, 4, 8, 12, 16, 20]
            for k in range(len(bounds) - 1):
                lo_, hi_ = bounds[k], bounds[k + 1]
                mseg = sml.tile([PPART, hi_ - lo_, 8], f32, name=f"mseg{k}")
                for a in range(lo_, hi_):
                    match_block(a, mseg, lo_)
                decode(lo_, hi_, k, mseg)
                gather(lo_, hi_)
                smooth(lo_, hi_, 2 * k, sub_eng=nc.gpsimd if k <= 1 else None)

            nc.sync.dma_start(out_d[:], partials[:])

    nc.finalize()
    return nc


def _prep_host(pred_past, pred_now, pad_loc, pad_loc_mask, pad_loc_target, n_pad):
    """Build all per-core host tensors (list of 8 dicts)."""
    n = pred_past.shape[1]
    nsh = n_pad // N_CORES
    A = nsh // PPART

    valid = (~pad_loc_mask).astype(np.float32)

    # full agent-major trajectories, zeroed outside valid agents
    lp = np.zeros((n_pad, NUM_MODES, TC), np.float32)
    ln = np.zeros((n_pad, NUM_MODES, TC), np.float32)
    pp = pred_past[..., :2].transpose(1, 0, 2, 3) + pad_loc.transpose(1, 0, 2)[
        :, :, None, :
    ]
    pn = pred_now[..., :2].transpose(1, 0, 2, 3) + pad_loc_target[:, None, None, :]
    pp *= valid[:, None, None, None]
    pn *= valid[:, None, None, None]
    lp[:n, :, 0:60] = pp.reshape(n, NUM_MODES, 60)
    ln[:n, :, 0:60] = pn.reshape(n, NUM_MODES, 60)

    # quantized endpoint distance matrix -> tds rows (f16, exact ints)
    qd = np.zeros((n_pad, 36), np.float16)
    dx = pp[:, :, None, T - 1, 0] - pn[:, None, :, T - 1, 0]
    dy = pp[:, :, None, T - 1, 1] - pn[:, None, :, T - 1, 1]
    dist = np.minimum(np.sqrt((dx * dx + dy * dy).astype(np.float32)), 8.0)
    qd[:n] = (1024.0 + 16.0 * dist.reshape(n, 36)).astype(np.float16)

    negs = _host_negs()
    cc = (
        384.0 * (np.arange(A, dtype=np.float32) % 2)[None, :]
        + 24.0 * (np.arange(PPART, dtype=np.float32) % 16)[:, None]
    )

    in_maps = []
    for c in range(N_CORES):
        s = slice(c * nsh, (c + 1) * nsh)
        lp_c = lp[s]  # [nsh, 6, 64]
        ln_c = ln[s]
        # [a, g, rp, j, s, e] -> [g, s, a, rp, j, e]
        src_ln = ln_c.reshape(A, 8, 16, NUM_MODES, 16, 4)
        lnT = src_ln.transpose(1, 4, 0, 2, 3, 5).reshape(PPART, -1)
        src_lp = lp_c.reshape(A, 8, 16, NUM_MODES, 16, 4)
        # [g, s, a, i, rp, e] so free offset = ((6a+i)*16 + rp)*4 + e
        lpT = src_lp.transpose(1, 4, 0, 3, 2, 5).reshape(PPART, -1)

        # tds [38, A*128] f16: rows 0-35 = quantized dists (transposed),
        # rows 36/37 = 1.0
        td = np.ones((38, A * PPART), np.float16)
        td[0:36] = qd[s].reshape(A * PPART, 36).T

        in_maps.append(
            {
                "tds": np.ascontiguousarray(td).view(np.uint16).view(np.float32),
                "negs": negs.view(np.float32),
                "cconst": cc,
                "lnT": _bf16_pack(lnT).view(np.float32),
                "lpT": _bf16_pack(lpT).view(np.float32),
            }
        )
    return in_maps, float(max(valid.sum(), 1.0)), A


_CACHE = {}
LAST_RESULT = None


def kernel(pred_past, pred_now, pad_loc, pad_loc_mask, pad_loc_target):
    global LAST_RESULT
    from concourse.bass_utils import run_bass_kernel_spmd

    pred_past = np.asarray(pred_past, np.float32)
    pred_now = np.asarray(pred_now, np.float32)
    pad_loc = np.asarray(pad_loc, np.float32)
    pad_loc_mask = np.asarray(pad_loc_mask, bool)
    pad_loc_target = np.asarray(pad_loc_target, np.float32)

    n = pred_past.shape[1]
    step = N_CORES * PPART * 2
    n_pad = ((n + step - 1) // step) * step
    nsh = n_pad // N_CORES

    in_maps, n_valid, A = _prep_host(
        pred_past, pred_now, pad_loc, pad_loc_mask, pad_loc_target, n_pad
    )

    if nsh not in _CACHE:
        _CACHE[nsh] = build_nc(nsh)
    nc = _CACHE[nsh]

    res = run_bass_kernel_spmd(nc, in_maps, list(range(N_CORES)))
    LAST_RESULT = res
    parts = np.stack([r["partials"] for r in res.results])  # [8, 128, ncols]
    sums = parts.sum(axis=(0, 1), dtype=np.float64)

    k_cons = N_CORES * PPART * (A * 16 * NUM_MODES * 4)
    cons_sum = sums[0::2].sum() - 0.5 * k_cons + 0.5 * sums[1::2].sum()
    cons_loss = np.float32(cons_sum / (NUM_MODES * T * 2 * n_valid))

    # reg loss is a cheap pure function of two small inputs -> host
    rd = (pad_loc.transpose(1, 0, 2) - pad_loc_target[:, None, :]) * (
        ~pad_loc_mask
    ).astype(np.float32)[:, None, None]
    ra = np.abs(rd)
    rr = np.maximum(1.0 - ra, 0.0)
    reg_sum = (
        ra.sum(dtype=np.float64)
        - 0.5 * rd.size
        + 0.5 * (rr.astype(np.float64) ** 2).sum()
    )
    reg_loss = np.float32(reg_sum / (NUM_MODES * 2 * n_valid))
    return (reg_loss, cons_loss)
